# revision 29
# baseline (speedup 1.0000x reference)
"""Trainium2 8-core Bass kernel for nn_AntisymmetricExpGenerator.

Reference computation (H=2048, B=512):
    A      = 0.5*(W - W.T)                      (antisymmetric)
    rec    = h @ expm(A*d).T
    b      = cat([du, u]) @ Bw.T
    M      = inv(A) @ (expm(A*d) - I)
    y      = (rec + b @ M.T) @ Cw.T

Key identity: inv(A) @ (expm(A*d) - I) = d * phi1(A*d) where
phi1(z) = (e^z - 1)/z = sum_k z^k/(k+1)!  is ENTIRE - no inverse and no
dense (H,H) expm/inv is needed.  With ||A*d|| ~ 8e-3 the series
converges after 2 terms (truncation ~1e-5 relative, far below the fp32
matmul noise of the reference itself):

    b @ M.T = d*(b + (d/4)*b@Abar.T + O(1e-5))        Abar = W - W.T
    rec     = h + (d/2)*h@Abar.T + O(3e-5)

so everything reduces to skinny matmuls of the batch block against
Abar - never a 2048^3 product.

Distribution (8 cores): H dim sharded 256 rows/core.  Activations live
transposed (feature on partitions, batch on free dim).  Weights are
pre-sliced per core on the host (layout only).  Two AllGather stages
(the [B0|h] block, then the H1 block) are each split into two
batch-column halves so collectives pipeline against the S1/C matmuls;
the first collective additionally hides part of the runtime's CC entry
barrier.

The h-vector chain rides as PSUM column 256 of the half-A matmuls
(N=257), so no separate mat-vec work exists.  The H1 AllGather ships
bf16 `inp = H1 - rec` (|inp| ~ 0.006 so bf16 error is suppressed 170x)
plus an exact hi/lo bf16 split of the f32 rec column; H1 is
reconstructed to float32r on-device.  Direct-path matmuls (Cw @ H1) run
float32r (TF32-like); end-to-end error vs reference ~2e-4.
"""

import sys

sys.path.insert(0, "/opt/trn_rl_repo")

import numpy as np
import ml_dtypes

import concourse.bass as bass
import concourse.mybir as mybir
import concourse.tile as tile
from concourse import bacc
from concourse.bass_utils import run_bass_kernel_spmd

# problem constants (hardcoded per harness contract)
DELTA = 0.01
B_SZ, U_DIM, DU_DIM, H_DIM, Y_DIM = 512, 1024, 512, 2048, 1024
F_DIM = U_DIM + DU_DIM  # 1536
N_CORES = 8
HS = H_DIM // N_CORES  # 256 rows of H per core
YS = Y_DIM // N_CORES  # 128 rows of y^T per core

F32 = mybir.dt.float32
F32R = mybir.dt.float32r
BF16 = mybir.dt.bfloat16
BF = ml_dtypes.bfloat16

P = 128
NB = B_SZ  # batch free dim (512)
NA = 256  # first batch half
NB2 = NB - NA  # second batch half
NBH = NA  # first-half width (legacy name used for half-A shapes)
KF = F_DIM // P  # 12 k-tiles for stage A
KH = H_DIM // P  # 16 k-tiles for H-contractions
MT = HS // P  # 2 m-tiles per core for H-sharded outputs
RG = [list(range(N_CORES))]


def _to_sb_layout(a: np.ndarray, dtype) -> np.ndarray:
    """(K, M) -> (128, (K//128)*M): k-tile kf lands at cols [kf*M,(kf+1)*M)."""
    K, M = a.shape
    assert K % P == 0
    return np.ascontiguousarray(
        a.reshape(K // P, P, M).transpose(1, 0, 2).reshape(P, (K // P) * M)
    ).astype(dtype, copy=False)


def build_nc():
    nc = bacc.Bacc("TRN2", target_bir_lowering=False, debug=False, num_devices=N_CORES)

    # --- per-core DRAM parameters (host-prepared layouts) ---
    catT = nc.dram_tensor("catT", [P, KF * NB], BF16, kind="ExternalInput")
    bwT = nc.dram_tensor("bwT", [P, KF * HS], BF16, kind="ExternalInput")
    wrowT = nc.dram_tensor("wrowT", [P, KH * HS], BF16, kind="ExternalInput")
    wcolN = nc.dram_tensor("wcolN", [P, KH * HS], BF16, kind="ExternalInput")
    cwTb = nc.dram_tensor("cwTb", [P, KH * YS], BF16, kind="ExternalInput")
    cwTf = nc.dram_tensor("cwTf", [P, KH * YS], F32, kind="ExternalInput")
    vcol = nc.dram_tensor("vcol", [P, MT], F32, kind="ExternalInput")

    out = nc.dram_tensor("out", [YS, NB], F32, kind="ExternalOutput")

    d = DELTA

    with tile.TileContext(nc) as tc:
        with (
            tc.tile_pool(name="wpool", bufs=1) as wpool,
            tc.tile_pool(name="acts", bufs=1) as apool,
            tc.tile_pool(name="psumA", bufs=2, space="PSUM") as psA,
            tc.tile_pool(name="psumM", bufs=4, space="PSUM") as psM,
            tc.tile_pool(name="psumC", bufs=2, space="PSUM") as psC,
            tc.tile_pool(name="dram", bufs=1, space="DRAM") as dram,
        ):
            # ---------- load inputs ----------
            # DMA *issue* on the sync sequencer costs ~0.6us per dma_start
            # regardless of size, so batch k-tiles into block transfers:
            # 2 blocks per tensor = dep granularity for an early start
            # without paying per-k-tile issue serialization.
            HKF = KF // 2  # 6
            HKH = KH // 2  # 8
            catT_sb = [
                apool.tile([P, HKF * NB], BF16, tag="catT", bufs=2, name=f"catT_sb{i}")
                for i in range(2)
            ]
            bwT_sb = [
                apool.tile([P, HKF * HS], BF16, tag="bwT", bufs=2, name=f"bwT_sb{i}")
                for i in range(2)
            ]
            wrowT_sb = [
                apool.tile([P, HKH * HS], BF16, tag="wrowT", bufs=2, name=f"wrowT_sb{i}")
                for i in range(2)
            ]
            wcolN_sb = [
                apool.tile([P, HKH * HS], BF16, tag="wcolN", bufs=2, name=f"wcolN_sb{i}")
                for i in range(2)
            ]
            cwTb_sb = [
                apool.tile([P, HKH * YS], BF16, tag="cwTb", bufs=2, name=f"cwTb_sb{i}")
                for i in range(2)
            ]
            cwTf_sb = [
                apool.tile([P, HKH * YS], F32, tag="cwTf", bufs=2, name=f"cwTf_sb{i}")
                for i in range(2)
            ]
            v_sb = wpool.tile([P, MT], F32)
            for i in range(2):
                nc.sync.dma_start(
                    catT_sb[i][:], catT[:, i * HKF * NB : (i + 1) * HKF * NB]
                )
                nc.sync.dma_start(
                    bwT_sb[i][:], bwT[:, i * HKF * HS : (i + 1) * HKF * HS]
                )
            nc.sync.dma_start(v_sb[:], vcol[:])
            for i in range(2):
                nc.sync.dma_start(
                    wrowT_sb[i][:], wrowT[:, i * HKH * HS : (i + 1) * HKH * HS]
                )
                nc.sync.dma_start(
                    wcolN_sb[i][:], wcolN[:, i * HKH * HS : (i + 1) * HKH * HS]
                )
            for i in range(2):
                nc.sync.dma_start(
                    cwTb_sb[i][:], cwTb[:, i * HKH * YS : (i + 1) * HKH * YS]
                )
                nc.sync.dma_start(
                    cwTf_sb[i][:], cwTf[:, i * HKH * YS : (i + 1) * HKH * YS]
                )

            def cat_k(kf):
                return catT_sb[kf // HKF][:, (kf % HKF) * NB : (kf % HKF + 1) * NB]

            def bw_k(kf, mi):
                base = (kf % HKF) * HS + mi * P
                return bwT_sb[kf // HKF][:, base : base + P]

            def wrow_k(kf, mi):
                base = (kf % HKH) * HS + mi * P
                return wrowT_sb[kf // HKH][:, base : base + P]

            def wcol_k(kf, mi):
                base = (kf % HKH) * HS + mi * P
                return wcolN_sb[kf // HKH][:, base : base + P]

            def cwb_k(kf):
                return cwTb_sb[kf // HKH][:, (kf % HKH) * YS : (kf % HKH + 1) * YS]

            def cwf_k(kf):
                return cwTf_sb[kf // HKH][:, (kf % HKH) * YS : (kf % HKH + 1) * YS]

            # ---------- stage A: B0[I_c] ----------
            pA_list = []
            z0_pay = []  # (128, 513): [B0 half A | v | B0 half B]
            for mi in range(MT):
                pA = psA.tile([P, NB], F32, tag="psA", name=f"pA{mi}")
                for kf in range(KF):
                    nc.tensor.matmul(
                        pA[:],
                        bw_k(kf, mi),
                        cat_k(kf),
                        start=(kf == 0),
                        stop=(kf == KF - 1),
                    )
                z0p = apool.tile([P, NB + 1], BF16, tag="z0p", bufs=2, name=f"z0p{mi}")
                nc.vector.tensor_copy(z0p[:, 0:NA], pA[:, 0:NA])
                nc.vector.tensor_copy(z0p[:, NA : NA + 1], v_sb[:, mi : mi + 1])
                nc.vector.tensor_copy(z0p[:, NA + 1 : NB + 1], pA[:, NA:NB])
                pA_list.append(pA)
                z0_pay.append(z0p)

            # ---------- AllGather Z0 (single op: [halfA | v | halfB]) ----
            ag0_in = dram.tile([HS, NB + 1], BF16)
            ag0_out = dram.tile([H_DIM, NB + 1], BF16, addr_space="Shared")
            for mi in range(MT):
                nc.gpsimd.dma_start(ag0_in[mi * P : (mi + 1) * P, :], z0_pay[mi][:])
            nc.gpsimd.collective_compute(
                "AllGather", mybir.AluOpType.bypass, replica_groups=RG,
                ins=[ag0_in.opt()], outs=[ag0_out.opt()],
            )
            # gathered -> SBUF in 4-k-tile blocks; ALL half-A blocks issued
            # before any half-B block (sync sequencer is FIFO - a half-B DMA
            # waiting on AG0b must not head-of-line-block half-A data).
            BLKS = [4, 4, 4, 4]  # k-tiles per gathered-DMA block
            BOFF = [0, 4, 8, 12]
            NBLK = len(BLKS)
            z0g_sb = [
                apool.tile(
                    [P, BLKS[i], NB + 1], BF16, tag=f"z0g{i}", bufs=1, name=f"z0g{i}"
                )
                for i in range(NBLK)
            ]
            for b in range(NBLK):
                nc.sync.dma_start(
                    z0g_sb[b][:],
                    ag0_out[BOFF[b] * P : (BOFF[b] + BLKS[b]) * P, :].rearrange(
                        "(k p) c -> p k c", p=P
                    ),
                )

            def blk_idx(kf):
                for b in range(NBLK - 1, -1, -1):
                    if kf >= BOFF[b]:
                        return b, kf - BOFF[b]
                raise AssertionError

            # ---------- stage S1: Z1[I_c] = Abar @ Z0, half A then half B ----
            pMa = []
            pMb = []
            for mi in range(MT):
                pMa.append(psM.tile([P, NBH + 1], F32, tag="psM", name=f"pMa{mi}"))
                pMb.append(psM.tile([P, NB2], F32, tag="psM", name=f"pMb{mi}"))
            for half in range(2):
                for mi in range(MT):
                    pM = (pMa if half == 0 else pMb)[mi]
                    lo, hi = (0, NA + 1) if half == 0 else (NA + 1, NB + 1)
                    n_mm = 0
                    for term_k in (wrow_k, wcol_k):
                        for kf in range(KH):
                            b, j = blk_idx(kf)
                            nc.tensor.matmul(
                                pM[:],
                                term_k(kf, mi),
                                z0g_sb[b][:, j, lo:hi],
                                start=(n_mm == 0),
                                stop=(n_mm == 2 * KH - 1),
                            )
                            n_mm += 1

            # ---------- combine ----------
            # rec_col = v + (d/2) Z1v  (exact f32, shipped as bf16 hi+lo)
            # inp     = d*B0 + (d^2/4) Z1   (bf16: |inp|~0.006, error suppressed)
            # Half A fully first so AG1a can fire while S1 half B still runs.
            z1a_pay = []
            z1b_pay = []
            cvs = []
            for mi in range(MT):
                cv = apool.tile([P, 1], F32, tag="cv", bufs=MT, name=f"cv{mi}")
                nc.scalar.activation(
                    cv[:],
                    pMa[mi][:, NBH : NBH + 1],
                    mybir.ActivationFunctionType.Identity,
                    bias=v_sb[:, mi : mi + 1],
                    scale=d / 2.0,
                )
                cvs.append(cv)
                paya = apool.tile(
                    [P, NBH + 2], BF16, tag="paya", bufs=MT, name=f"paya{mi}"
                )
                # hi/lo split of cv into payload cols 256/257
                hi_f = apool.tile([P, 1], F32, tag="hi_f", bufs=MT, name=f"hi_f{mi}")
                nc.vector.tensor_copy(paya[:, NBH : NBH + 1], cv[:])  # f32->bf16 round
                nc.vector.tensor_copy(hi_f[:], paya[:, NBH : NBH + 1])  # back to f32
                nc.vector.tensor_sub(paya[:, NBH + 1 : NBH + 2], cv[:], hi_f[:])
                t = apool.tile([P, NBH], F32, tag="t", bufs=2 * MT, name=f"ta{mi}")
                nc.scalar.activation(
                    t[:],
                    pMa[mi][:, 0:NBH],
                    mybir.ActivationFunctionType.Identity,
                    bias=0.0,
                    scale=d * d / 4.0,
                )
                nc.vector.scalar_tensor_tensor(
                    paya[:, 0:NBH],
                    pA_list[mi][:, 0:NBH],
                    d,
                    t[:],
                    op0=mybir.AluOpType.mult,
                    op1=mybir.AluOpType.add,
                )
                z1a_pay.append(paya)

            # AG1a fires here (only depends on half A)
            ag1a_in = dram.tile([HS, NBH + 2], BF16)
            ag1a_out = dram.tile([H_DIM, NBH + 2], BF16, addr_space="Shared")
            for mi in range(MT):
                nc.gpsimd.dma_start(ag1a_in[mi * P : (mi + 1) * P, :], z1a_pay[mi][:])
            nc.gpsimd.collective_compute(
                "AllGather", mybir.AluOpType.bypass, replica_groups=RG,
                ins=[ag1a_in.opt()], outs=[ag1a_out.opt()],
            )

            for mi in range(MT):
                payb = apool.tile([P, NB2], BF16, tag="payb", bufs=MT, name=f"payb{mi}")
                t = apool.tile([P, NB2], F32, tag="t", bufs=2 * MT, name=f"tb{mi}")
                nc.scalar.activation(
                    t[:],
                    pMb[mi][:, 0:NB2],
                    mybir.ActivationFunctionType.Identity,
                    bias=0.0,
                    scale=d * d / 4.0,
                )
                nc.vector.scalar_tensor_tensor(
                    payb[:],
                    pA_list[mi][:, NBH:NB],
                    d,
                    t[:],
                    op0=mybir.AluOpType.mult,
                    op1=mybir.AluOpType.add,
                )
                z1b_pay.append(payb)

            ag1b_in = dram.tile([HS, NB2], BF16)
            ag1b_out = dram.tile([H_DIM, NB2], BF16, addr_space="Shared")
            for mi in range(MT):
                nc.gpsimd.dma_start(ag1b_in[mi * P : (mi + 1) * P, :], z1b_pay[mi][:])
            nc.gpsimd.collective_compute(
                "AllGather", mybir.AluOpType.bypass, replica_groups=RG,
                ins=[ag1b_in.opt()], outs=[ag1b_out.opt()],
            )

            # ---------- stage C: yT[J_c] = Cw @ inp  +  (Cw @ rec) rank-1 ----
            y_sb = apool.tile([P, NB], F32, tag="y", name="y_sb")
            pR = psA.tile([P, 1], F32, tag="psA", name="pR")  # reuses freed pA slot
            for half in range(2):
                ag_out = ag1a_out if half == 0 else ag1b_out
                w = NA + 2 if half == 0 else NB2
                g_blk = [
                    apool.tile(
                        [P, BLKS[b], w], BF16, tag=f"g{b}", bufs=2, name=f"g{half}_{b}"
                    )
                    for b in range(NBLK)
                ]
                for b in range(NBLK):
                    nc.sync.dma_start(
                        g_blk[b][:],
                        ag_out[BOFF[b] * P : (BOFF[b] + BLKS[b]) * P, :].rearrange(
                            "(k p) c -> p k c", p=P
                        ),
                    )
                hw = NA if half == 0 else NB2
                pC = psC.tile([P, hw], F32, tag="psC", name=f"pC{half}")
                for kf in range(KH):
                    b, j = blk_idx(kf)
                    g = g_blk[b][:, j]
                    if half == 0:
                        # rec column: exact f32 from hi+lo bf16 cols
                        rec_col = apool.tile(
                            [P, 1], F32, tag="rec", bufs=KH, name=f"rec{kf}"
                        )
                        nc.vector.tensor_add(
                            rec_col[:], g[:, NBH : NBH + 1], g[:, NBH + 1 : NBH + 2]
                        )
                        # rank-1 part: yrec[J_c] += Cw_kf^T (f32) @ rec_kf
                        nc.tensor.matmul(
                            pR[:],
                            cwf_k(kf),
                            rec_col[:],
                            start=(kf == 0),
                            stop=(kf == KH - 1),
                        )
                    nc.tensor.matmul(
                        pC[:],
                        cwb_k(kf),
                        g[:, 0:hw],
                        start=(kf == 0),
                        stop=(kf == KH - 1),
                    )
                # y = inp-part + broadcast rec-part
                nc.vector.tensor_scalar(
                    y_sb[:, half * NA : half * NA + hw],
                    pC[:],
                    pR[:],
                    None,
                    op0=mybir.AluOpType.add,
                )
                nc.sync.dma_start(
                    out[:, half * NA : half * NA + hw],
                    y_sb[:, half * NA : half * NA + hw],
                )

    nc.compile()
    return nc


_NC_CACHE = None


def _get_nc():
    global _NC_CACHE
    if _NC_CACHE is None:
        _NC_CACHE = build_nc()
    return _NC_CACHE


def make_in_maps(u, du, W, Bw, Cw, h):
    cat = np.concatenate([du, u], axis=1)  # (B, F)
    catT = _to_sb_layout(np.ascontiguousarray(cat.T), BF)
    in_maps = []
    for c in range(N_CORES):
        sl = slice(c * HS, (c + 1) * HS)
        ysl = slice(c * YS, (c + 1) * YS)
        in_maps.append(
            {
                "catT": catT,
                "bwT": _to_sb_layout(np.ascontiguousarray(Bw[sl, :].T), BF),
                "wrowT": _to_sb_layout(np.ascontiguousarray(W[sl, :].T), BF),
                "wcolN": _to_sb_layout(np.ascontiguousarray(-W[:, sl]), BF),
                "cwTb": _to_sb_layout(np.ascontiguousarray(Cw[ysl, :].T), BF),
                "cwTf": _to_sb_layout(np.ascontiguousarray(Cw[ysl, :].T), np.float32),
                "vcol": np.ascontiguousarray(
                    h[0, sl].reshape(MT, P).T, dtype=np.float32
                ),
            }
        )
    return in_maps


def kernel(u, du, W, Bw, Cw, h):
    u = np.asarray(u, dtype=np.float32)
    du = np.asarray(du, dtype=np.float32)
    W = np.asarray(W, dtype=np.float32)
    Bw = np.asarray(Bw, dtype=np.float32)
    Cw = np.asarray(Cw, dtype=np.float32)
    h = np.asarray(h, dtype=np.float32)

    in_maps = make_in_maps(u, du, W, Bw, Cw, h)
    nc = _get_nc()
    res = run_bass_kernel_spmd(nc, in_maps, core_ids=list(range(N_CORES)))
    yT = np.concatenate([res.results[c]["out"] for c in range(N_CORES)], axis=0)
    return np.ascontiguousarray(yT.T)
